# revision 9
# baseline (speedup 1.0000x reference)
"""ANI AEV kernel on 8 TRN2 NeuronCores (Bass/Tile).

v2 strategy (atom/segment-partitioned, no collectives):
 - radial segments  = edge_src*4  + species[edge_dst]   in [0, 160000)
 - angular segments = central_atom*10 + pair            in [0, 400000)
 - each core owns a contiguous 1/8 of the segments. Host routes + sorts
   edges/triples to the owning core. Each segment gets J1 dense "plane"
   slots (first J1 edges, zero-padded); rare overflow edges (count > J1)
   go to a per-128-seg-window one-hot stream.
 - device: elementwise term math on ScalarE/VectorE in a [seg-partition,
   (r, window, j)] layout; plane summation via TensorE matmuls with a
   constant identity lhsT over whole 16-window chunks (psum += plane_j);
   overflow via per-tile is_equal one-hot + matmul.
 - outputs are written in a device-friendly [128, W*16] staging layout;
   host transposes and assembles the [40000, 224] result.
"""

import os
import sys

sys.path.insert(0, "/opt/trn_rl_repo")

from contextlib import ExitStack

import ml_dtypes
import numpy as np

import concourse.bass as bass
import concourse.tile as tile
from concourse import bacc, mybir
from concourse.bass_utils import run_bass_kernel_spmd

LAST_EXEC_NS = None

# ---- problem constants (hardcoded) ----
NCORES = 8
N = 40000
NSP = 4
RDIV = 16
ADIV = 4
ASEC = 4
RETA = 16.0
AETA = 8.0
ZETA = 32.0
CUT = 5.2
ACUT = 3.5
RSTART = 0.8
ASTART = 0.8
NPAIR = NSP * (NSP + 1) // 2  # 10

SR = N * NSP  # 160000 radial segments
SA = N * NPAIR  # 400000 angular segments
SEG_R = SR // NCORES  # 20000 per core
SEG_A = SA // NCORES  # 50000 per core
WINR = (SEG_R + 127) // 128  # 157
WINA = (SEG_A + 127) // 128  # 391

J1R = int(os.environ.get("AEV_J1R", "24"))
J1A = int(os.environ.get("AEV_J1A", "16"))

F32 = mybir.dt.float32
BF16 = mybir.dt.bfloat16

SHIFT_R = (-np.linspace(RSTART, CUT, RDIV + 1)[:-1]).astype(np.float64)
SHIFT_Z = (-(np.linspace(0, np.pi, ASEC + 1) + np.pi / (2 * ASEC))[:-1]).astype(
    np.float64
)
SHIFT_A = (-np.sqrt(AETA) * np.linspace(ASTART, ACUT, ADIV + 1)[:-1]).astype(np.float64)


def _triu_table():
    s1, s2 = np.triu_indices(NSP, 0)
    t = np.zeros((NSP, NSP), dtype=np.int64)
    t[s1, s2] = np.arange(s1.shape[0])
    t[s2, s1] = t[s1, s2]
    return t


def _plan_slots2(seg, seg_per_core, nwin, j1, payloads, fills):
    """Split items into per-segment plane slots (rank < j1) + window overflow.

    Returns (plane_arrs [8,128,nwin*j1], ovf_arrs, ovf_ix, kov).
    """
    seg = seg.astype(np.int64)
    core = seg // seg_per_core
    local = seg - core * seg_per_core
    w = local >> 7
    m = local & 127

    nseg_tot = NCORES * seg_per_core
    counts = np.bincount(seg, minlength=nseg_tot)
    starts = np.zeros(nseg_tot + 1, np.int64)
    np.cumsum(counts, out=starts[1:])
    order = np.argsort(seg, kind="stable")
    rank = np.arange(seg.shape[0], dtype=np.int64) - starts[seg[order]]

    core_s = core[order]
    w_s = w[order]
    m_s = m[order]

    plane = rank < j1
    parrs = []
    for vals, fill in zip(payloads, fills):
        a = np.full((NCORES, 128, nwin * j1), fill, np.float32)
        a[core_s[plane], m_s[plane], w_s[plane] * j1 + rank[plane]] = vals[order][plane]
        parrs.append(a)

    # overflow: pack per (core, window)
    ovf = ~plane
    gw = (core_s * nwin + w_s)[ovf]
    ocounts = np.bincount(gw, minlength=NCORES * nwin)
    kov = max(1, int((ocounts.max() + 127) // 128))
    ostarts = np.zeros(NCORES * nwin + 1, np.int64)
    np.cumsum(ocounts, out=ostarts[1:])
    oorder = np.argsort(gw, kind="stable")
    opos = np.arange(gw.shape[0], dtype=np.int64) - ostarts[gw[oorder]]
    oc = core_s[ovf][oorder]
    ow = w_s[ovf][oorder]
    om = m_s[ovf][oorder]
    ocol = ow * kov + (opos >> 7)
    op_ = opos & 127

    oarrs = []
    for vals, fill in zip(payloads, fills):
        a = np.full((NCORES, 128, nwin * kov), fill, np.float32)
        a[oc, op_, ocol] = vals[order][ovf][oorder]
        oarrs.append(a)
    oix = np.zeros((NCORES, 128, nwin * kov), np.float32)
    oix[oc, op_, ocol] = om.astype(np.float32)
    return parrs, oarrs, oix, kov


def _build_nc(kovr, kova):
    """Build the SPMD Bass graph (identical on all cores)."""
    TPR = WINR * J1R  # radial plane cols
    TPA = WINA * J1A
    TOR = WINR * kovr  # overflow cols
    TOA = WINA * kova
    nc = bacc.Bacc(None, target_bir_lowering=False, debug=False, num_devices=NCORES)

    rp_d = nc.declare_dram_parameter("rp_d", [128, TPR], F32, isOutput=False)
    rp_sw = nc.declare_dram_parameter("rp_sw", [128, TPR], F32, isOutput=False)
    ro_d = nc.declare_dram_parameter("ro_d", [128, TOR], F32, isOutput=False)
    ro_sw = nc.declare_dram_parameter("ro_sw", [128, TOR], F32, isOutput=False)
    ro_ix = nc.declare_dram_parameter("ro_ix", [128, TOR], F32, isOutput=False)
    ap_th = nc.declare_dram_parameter("ap_th", [128, TPA], F32, isOutput=False)
    ap_dd = nc.declare_dram_parameter("ap_dd", [128, TPA], F32, isOutput=False)
    ap_f1 = nc.declare_dram_parameter("ap_f1", [128, TPA], F32, isOutput=False)
    ao_th = nc.declare_dram_parameter("ao_th", [128, TOA], F32, isOutput=False)
    ao_dd = nc.declare_dram_parameter("ao_dd", [128, TOA], F32, isOutput=False)
    ao_f1 = nc.declare_dram_parameter("ao_f1", [128, TOA], F32, isOutput=False)
    ao_ix = nc.declare_dram_parameter("ao_ix", [128, TOA], F32, isOutput=False)
    iota_p = nc.declare_dram_parameter("iota", [128, 128], BF16, isOutput=False)
    ident_p = nc.declare_dram_parameter("ident", [128, 128], BF16, isOutput=False)
    out_r = nc.declare_dram_parameter("out_r", [128, WINR * 16], F32, isOutput=True)
    out_a = nc.declare_dram_parameter("out_a", [128, WINA * 16], F32, isOutput=True)

    AF = mybir.ActivationFunctionType
    OP = mybir.AluOpType

    def sview(ap, off, dims):
        # strided free-dim view of a [128, X] tile AP
        return bass.AP(ap.tensor, ap.offset + off, [list(ap.ap[0])] + dims)

    with tile.TileContext(nc) as tc, ExitStack() as ctx:
        const = ctx.enter_context(tc.tile_pool(name="const", bufs=1))
        io = ctx.enter_context(tc.tile_pool(name="io", bufs=2))
        work = ctx.enter_context(tc.tile_pool(name="work", bufs=2))
        ohp = ctx.enter_context(tc.tile_pool(name="ohp", bufs=4))
        psp = ctx.enter_context(tc.tile_pool(name="psp", bufs=2, space="PSUM"))
        outp = ctx.enter_context(tc.tile_pool(name="outp", bufs=2))

        iota_t = const.tile([128, 128], BF16)
        nc.sync.dma_start(out=iota_t[:], in_=iota_p[:])
        ident_t = const.tile([128, 128], BF16)
        nc.sync.dma_start(out=ident_t[:], in_=ident_p[:])

        cvals = [0.0, 1.0] + [float(SHIFT_Z[z] + np.pi / 2) for z in range(ASEC)]
        cb = const.tile([128, len(cvals)], F32)
        for i, v in enumerate(cvals):
            nc.vector.memset(cb[:, i : i + 1], v)
            nc.const_aps.aps[(F32, v)] = cb[:, i : i + 1]

        G = 16  # windows per chunk

        # ---------------- radial ----------------
        for c0 in range(0, WINR, G):
            g = min(G, WINR - c0)
            C = g * J1R
            Co = g * kovr
            d_t = io.tile([128, C], F32, tag="rp_d")
            sw_t = io.tile([128, C], F32, tag="rp_sw")
            nc.sync.dma_start(out=d_t[:], in_=rp_d[:, c0 * J1R : c0 * J1R + C])
            nc.sync.dma_start(out=sw_t[:], in_=rp_sw[:, c0 * J1R : c0 * J1R + C])
            od_t = io.tile([128, Co], F32, tag="ro_d")
            osw_t = io.tile([128, Co], F32, tag="ro_sw")
            oix_t = io.tile([128, Co], F32, tag="ro_ix")
            nc.sync.dma_start(out=od_t[:], in_=ro_d[:, c0 * kovr : c0 * kovr + Co])
            nc.sync.dma_start(out=osw_t[:], in_=ro_sw[:, c0 * kovr : c0 * kovr + Co])
            nc.sync.dma_start(out=oix_t[:], in_=ro_ix[:, c0 * kovr : c0 * kovr + Co])

            x = work.tile([128, RDIV * C], F32, tag="x")
            for r in range(RDIV):
                nc.vector.tensor_scalar_add(
                    x[:, r * C : (r + 1) * C], d_t[:], float(SHIFT_R[r])
                )
            sq = work.tile([128, RDIV * C], F32, tag="sq")
            nc.scalar.activation(sq[:], x[:], AF.Square)
            ex = work.tile([128, RDIV * C], F32, tag="x")
            nc.scalar.activation(ex[:], sq[:], AF.Exp, scale=-RETA)
            tm = work.tile([128, RDIV * C], BF16, tag="tm")
            for r in range(RDIV):
                nc.vector.tensor_tensor(
                    tm[:, r * C : (r + 1) * C],
                    ex[:, r * C : (r + 1) * C],
                    sw_t[:],
                    op=OP.mult,
                )
            # overflow terms (r-major per-tile layout)
            xo = work.tile([128, RDIV * Co], F32, tag="xo")
            for r in range(RDIV):
                nc.vector.tensor_scalar_add(
                    xo[:, r * Co : (r + 1) * Co], od_t[:], float(SHIFT_R[r])
                )
            sqo = work.tile([128, RDIV * Co], F32, tag="sqo")
            nc.scalar.activation(sqo[:], xo[:], AF.Square)
            exo = work.tile([128, RDIV * Co], F32, tag="xo")
            nc.scalar.activation(exo[:], sqo[:], AF.Exp, scale=-RETA)
            tmo = work.tile([128, RDIV * Co], BF16, tag="tmo")
            for r in range(RDIV):
                nc.vector.tensor_tensor(
                    tmo[:, r * Co : (r + 1) * Co],
                    exo[:, r * Co : (r + 1) * Co],
                    osw_t[:],
                    op=OP.mult,
                )

            pt = psp.tile([128, g * 16], F32, tag="ps")
            for j in range(J1R):
                nc.tensor.matmul(
                    pt[:, : g * 16],
                    lhsT=ident_t[:],
                    rhs=sview(tm[:, :], j, [[J1R, g], [C, 16]]),
                    start=(j == 0),
                    stop=False,
                    skip_group_check=True,
                )
            for t in range(Co):
                wi = t // kovr
                o = ohp.tile([128, 128], BF16, tag="oh")
                nc.vector.tensor_scalar(
                    o[:], iota_t[:], oix_t[:, t : t + 1], None, op0=OP.is_equal
                )
                nc.tensor.matmul(
                    pt[:, wi * 16 : (wi + 1) * 16],
                    lhsT=o[:],
                    rhs=sview(tmo[:, :], t, [[Co, 16]]),
                    start=False,
                    stop=(t == Co - 1),
                    skip_group_check=True,
                )
            st = outp.tile([128, g * 16], F32, tag="st")
            nc.scalar.activation(st[:], pt[:], AF.Copy)
            nc.sync.dma_start(out=out_r[:, c0 * 16 : (c0 + g) * 16], in_=st[:])

        # ---------------- angular ----------------
        for c0 in range(0, WINA, G):
            g = min(G, WINA - c0)
            C = g * J1A
            Co = g * kova
            th_t = io.tile([128, C], F32, tag="ap_th")
            dd_t = io.tile([128, C], F32, tag="ap_dd")
            f1_t = io.tile([128, C], F32, tag="ap_f1")
            nc.sync.dma_start(out=th_t[:], in_=ap_th[:, c0 * J1A : c0 * J1A + C])
            nc.sync.dma_start(out=dd_t[:], in_=ap_dd[:, c0 * J1A : c0 * J1A + C])
            nc.sync.dma_start(out=f1_t[:], in_=ap_f1[:, c0 * J1A : c0 * J1A + C])
            oth_t = io.tile([128, Co], F32, tag="ao_th")
            odd_t = io.tile([128, Co], F32, tag="ao_dd")
            of1_t = io.tile([128, Co], F32, tag="ao_f1")
            oix_t = io.tile([128, Co], F32, tag="ao_ix")
            nc.sync.dma_start(out=oth_t[:], in_=ao_th[:, c0 * kova : c0 * kova + Co])
            nc.sync.dma_start(out=odd_t[:], in_=ao_dd[:, c0 * kova : c0 * kova + Co])
            nc.sync.dma_start(out=of1_t[:], in_=ao_f1[:, c0 * kova : c0 * kova + Co])
            nc.sync.dma_start(out=oix_t[:], in_=ao_ix[:, c0 * kova : c0 * kova + Co])

            def ang_terms(thv, ddv, f1v, w, tag):
                # factor1 = f1*(1+cos(th+sz))^32 via exp(32*ln1p(cos)); cos = sin(x+pi/2)
                s1 = work.tile([128, ASEC * w], F32, tag=tag + "s1")
                for z in range(ASEC):
                    nc.scalar.activation(
                        s1[:, z * w : (z + 1) * w],
                        thv[:],
                        AF.Sin,
                        bias=float(SHIFT_Z[z] + np.pi / 2),
                    )
                L = work.tile([128, ASEC * w], F32, tag=tag + "L")
                nc.scalar.activation(L[:], s1[:], AF.Ln, bias=1.0)
                E = work.tile([128, ASEC * w], F32, tag=tag + "s1")
                nc.scalar.activation(E[:], L[:], AF.Exp, scale=float(ZETA))
                F1 = work.tile([128, ASEC * w], BF16, tag=tag + "F1")
                for z in range(ASEC):
                    nc.vector.tensor_tensor(
                        F1[:, z * w : (z + 1) * w],
                        E[:, z * w : (z + 1) * w],
                        f1v[:],
                        op=OP.mult,
                    )
                y = work.tile([128, ADIV * w], F32, tag=tag + "L")
                for a in range(ADIV):
                    nc.vector.tensor_scalar_add(
                        y[:, a * w : (a + 1) * w], ddv[:], float(SHIFT_A[a])
                    )
                ys = work.tile([128, ADIV * w], F32, tag=tag + "ys")
                nc.scalar.activation(ys[:], y[:], AF.Square)
                F2 = work.tile([128, ADIV * w], BF16, tag=tag + "L")
                nc.scalar.activation(F2[:], ys[:], AF.Exp, scale=-1.0)
                t_ = work.tile([128, ADIV * ASEC * w], BF16, tag=tag + "tm")
                for a in range(ADIV):
                    for z in range(ASEC):
                        nc.vector.tensor_tensor(
                            t_[:, (a * ASEC + z) * w : (a * ASEC + z + 1) * w],
                            F1[:, z * w : (z + 1) * w],
                            F2[:, a * w : (a + 1) * w],
                            op=OP.mult,
                        )
                return t_

            tm = ang_terms(th_t, dd_t, f1_t, C, "p")
            tmo = ang_terms(oth_t, odd_t, of1_t, Co, "o")

            pt = psp.tile([128, g * 16], F32, tag="ps")
            for j in range(J1A):
                nc.tensor.matmul(
                    pt[:, : g * 16],
                    lhsT=ident_t[:],
                    rhs=sview(tm[:, :], j, [[J1A, g], [C, 16]]),
                    start=(j == 0),
                    stop=False,
                    skip_group_check=True,
                )
            for t in range(Co):
                wi = t // kova
                o = ohp.tile([128, 128], BF16, tag="oh")
                nc.vector.tensor_scalar(
                    o[:], iota_t[:], oix_t[:, t : t + 1], None, op0=OP.is_equal
                )
                nc.tensor.matmul(
                    pt[:, wi * 16 : (wi + 1) * 16],
                    lhsT=o[:],
                    rhs=sview(tmo[:, :], t, [[Co, 16]]),
                    start=False,
                    stop=(t == Co - 1),
                    skip_group_check=True,
                )
            st = outp.tile([128, g * 16], F32, tag="st")
            nc.scalar.activation(st[:], pt[:], AF.Copy)
            nc.sync.dma_start(out=out_a[:, c0 * 16 : (c0 + g) * 16], in_=st[:])

    nc.compile()
    return nc


def _host_prep(species, rad_distances, rad_switch, edge_src, edge_dst,
               ang_distances, ang_switch, angles, central_atom,
               angle_src, angle_dst, ang_edge_dst):
    species = species.astype(np.int64)
    rseg = edge_src.astype(np.int64) * NSP + species[edge_dst.astype(np.int64)]
    rplane, rovf, rix, kovr = _plan_slots2(
        rseg,
        SEG_R,
        WINR,
        J1R,
        [rad_distances.astype(np.float32), 0.25 * rad_switch.astype(np.float32)],
        [1.0, 0.0],
    )
    triu = _triu_table()
    idx_dest = species[ang_edge_dst.astype(np.int64)]
    asrc = angle_src.astype(np.int64)
    adst = angle_dst.astype(np.int64)
    aseg = central_atom.astype(np.int64) * NPAIR + triu[idx_dest[asrc], idx_dest[adst]]
    dist = (0.5 * np.sqrt(AETA) * ang_distances).astype(np.float32)
    sw = (ang_switch * np.sqrt(2.0 * 0.5**ZETA)).astype(np.float32)
    d12 = dist[asrc] + dist[adst]
    f1 = sw[asrc] * sw[adst]
    aplane, aovf, aix, kova = _plan_slots2(
        aseg, SEG_A, WINA, J1A, [angles.astype(np.float32), d12, f1], [0.0, 1.0, 0.0]
    )
    return (rplane, rovf, rix, kovr), (aplane, aovf, aix, kova)


def _install_ntff_shim():
    """Register the terminal-side NTFF profiling hook (missing antenv.axon_hooks)."""
    import contextlib
    import ctypes
    import types

    import concourse.bass_utils as bu

    bu.upload_artifacts = lambda d: d  # no remote bucket in this container
    if "antenv.axon_hooks" in sys.modules:
        return
    lib = ctypes.CDLL("/opt/axon/libaxon_pjrt.so")
    if not hasattr(lib, "axon_start_nrt_profile"):
        return
    lib.axon_start_nrt_profile.argtypes = [
        ctypes.POINTER(ctypes.c_int64),
        ctypes.c_size_t,
    ]
    lib.axon_start_nrt_profile.restype = ctypes.c_int64
    lib.axon_stop_nrt_profile.argtypes = [ctypes.c_char_p]
    lib.axon_stop_nrt_profile.restype = ctypes.c_int64

    @contextlib.contextmanager
    def _hook(output_dir, device_ids):
        import jax

        jax.devices()
        if device_ids:
            ids = (ctypes.c_int64 * len(device_ids))(*device_ids)
            rc = lib.axon_start_nrt_profile(ids, len(device_ids))
        else:
            rc = lib.axon_start_nrt_profile(None, 0)
        if rc != 0:
            raise RuntimeError(f"axon_start_nrt_profile rc={rc}")
        try:
            yield
        finally:
            n = lib.axon_stop_nrt_profile(str(output_dir).encode())
            print(f"ntff profile: {n} file(s) -> {output_dir}", file=sys.stderr)

    mod = types.ModuleType("antenv.axon_hooks")
    mod.get_axon_ntff_profile_hook = lambda: _hook
    sys.modules["antenv.axon_hooks"] = mod


def kernel(**inputs):
    (rplane, rovf, rix, kovr), (aplane, aovf, aix, kova) = _host_prep(**inputs)

    nc = _build_nc(kovr, kova)

    iota = np.broadcast_to(
        np.arange(128, dtype=np.float32)[None, :], (128, 128)
    ).astype(ml_dtypes.bfloat16)
    ident = np.eye(128, dtype=np.float32).astype(ml_dtypes.bfloat16)
    in_maps = []
    for c in range(NCORES):
        in_maps.append(
            {
                "rp_d": np.ascontiguousarray(rplane[0][c]),
                "rp_sw": np.ascontiguousarray(rplane[1][c]),
                "ro_d": np.ascontiguousarray(rovf[0][c]),
                "ro_sw": np.ascontiguousarray(rovf[1][c]),
                "ro_ix": np.ascontiguousarray(rix[c]),
                "ap_th": np.ascontiguousarray(aplane[0][c]),
                "ap_dd": np.ascontiguousarray(aplane[1][c]),
                "ap_f1": np.ascontiguousarray(aplane[2][c]),
                "ao_th": np.ascontiguousarray(aovf[0][c]),
                "ao_dd": np.ascontiguousarray(aovf[1][c]),
                "ao_f1": np.ascontiguousarray(aovf[2][c]),
                "ao_ix": np.ascontiguousarray(aix[c]),
                "iota": iota,
                "ident": ident,
            }
        )

    trace = bool(os.environ.get("AEV_TRACE"))
    if trace:
        _install_ntff_shim()
    res = run_bass_kernel_spmd(
        nc, in_maps, list(range(NCORES)), trace=trace,
        tmpdir=os.environ.get("AEV_TRACE_DIR") or None,
    )
    results = res.results
    global LAST_EXEC_NS
    LAST_EXEC_NS = res.exec_time_ns

    rad = np.concatenate(
        [
            results[c]["out_r"]
            .reshape(128, WINR, 16)
            .transpose(1, 0, 2)
            .reshape(WINR * 128, 16)[:SEG_R]
            for c in range(NCORES)
        ],
        axis=0,
    ).reshape(N, NSP * RDIV)
    ang = np.concatenate(
        [
            results[c]["out_a"]
            .reshape(128, WINA, 16)
            .transpose(1, 0, 2)
            .reshape(WINA * 128, 16)[:SEG_A]
            for c in range(NCORES)
        ],
        axis=0,
    ).reshape(N, NPAIR * ADIV * ASEC)
    return np.concatenate([rad, ang], axis=-1).astype(np.float32)
